# revision 4
# baseline (speedup 1.0000x reference)
"""Trainium2 Bass kernel for EnhancedTrajectoryPredictor GNN message passing.

Data-parallel over batch: core c handles batch element c (T=4 windows each).
Per window-layer, the O(N^2*H) pairwise relu tensors are produced on the PE
via identity/ones broadcast matmuls, relu-drained on ACT, and consumed by
DVE (attention logits) / per-i PE matmuls (message aggregation), exploiting
softmax-row-sum==1 to eliminate the O(N^2*H*H) matmul.
"""
import numpy as np
import ml_dtypes

import concourse.bass as bass
import concourse.mybir as mybir
import concourse.tile as tile
from concourse import bacc
from concourse.bass_types import AP

F32 = mybir.dt.float32
BF16 = mybir.dt.bfloat16

B, N, T, IN = 8, 128, 4, 45
F, H, L = 64, 128, 3
NCORES = 8
CHUNK = 512                # psum chunk free size (j-block of 4)
NCHUNK = (N * H) // CHUNK  # 32

_bf = lambda x: np.ascontiguousarray(x).astype(ml_dtypes.bfloat16)
_f32 = lambda x: np.ascontiguousarray(x).astype(np.float32)

_CACHED_NC = None


def _build_nc():
    nc = bacc.Bacc(None, target_bir_lowering=False, debug=False)

    dp = lambda name, shape, dt: nc.declare_dram_parameter(name, list(shape), dt, isOutput=False)

    d_xT = dp("xT", (T, IN, N), BF16)
    d_mask = dp("maskadd", (N, N), F32)
    d_eye = dp("eye", (128, 128), BF16)
    d_eyef = dp("eyef", (128, 128), F32)
    d_ones = dp("ones1", (1, 128), BF16)
    d_wp = dp("wp", (IN, F), BF16)
    d_wo = dp("wo", (F, F), BF16)
    d_bp = dp("b_p", (F, 1), F32)
    d_bo = dp("b_o", (F, 1), F32)
    d_m1t = [dp(f"w_m1t{l}", (F, H), BF16) for l in range(L)]
    d_m1b = [dp(f"w_m1b{l}", (F, H), BF16) for l in range(L)]
    d_a1t = [dp(f"w_a1t{l}", (F, H), BF16) for l in range(L)]
    d_a1b = [dp(f"w_a1b{l}", (F, H), BF16) for l in range(L)]
    d_m2 = [dp(f"w_m2_{l}", (H, H), BF16) for l in range(L)]
    d_u1t = [dp(f"w_u1t{l}", (F, H), BF16) for l in range(L)]
    d_u1b = [dp(f"w_u1b{l}", (H, H), BF16) for l in range(L)]
    d_u2 = [dp(f"w_u2_{l}", (H, F), BF16) for l in range(L)]
    d_wa2r = [dp(f"wa2r{l}", (128, H), BF16) for l in range(L)]
    d_bm1 = [dp(f"b_m1_{l}", (H, 1), F32) for l in range(L)]
    d_ba1 = [dp(f"b_a1_{l}", (H, 1), F32) for l in range(L)]
    d_bu1 = [dp(f"b_u1_{l}", (H, 1), F32) for l in range(L)]
    d_bu2 = [dp(f"b_u2_{l}", (F, 1), F32) for l in range(L)]

    d_out = nc.declare_dram_parameter("out", [T, N, F], F32, isOutput=True)

    RELU = mybir.ActivationFunctionType.Relu
    EXP = mybir.ActivationFunctionType.Exp
    ADD = mybir.AluOpType.add
    MULT = mybir.AluOpType.mult
    MAX = mybir.AluOpType.max
    AX = mybir.AxisListType.X

    with tile.TileContext(nc) as tc:
        with (
            tc.tile_pool(name="wts", bufs=1) as wts,
            tc.tile_pool(name="sm", bufs=3) as sm,
            tc.tile_pool(name="big", bufs=1) as big,
            tc.tile_pool(name="fl", bufs=1) as fl,
            tc.tile_pool(name="ps", bufs=3, space="PSUM") as ps,
            tc.tile_pool(name="ps2", bufs=2, space="PSUM") as ps2,
            tc.tile_pool(name="psp", bufs=3, space="PSUM") as psp,
        ):
            # ---- load static tiles
            def ld(d, shape, dt):
                t = wts.tile(list(shape), dt, tag=f"w_{d.name}")
                nc.sync.dma_start(t[:], d[:])
                return t

            t_mask = ld(d_mask, (N, N), F32)
            t_eye = ld(d_eye, (128, 128), BF16)
            t_eyef = ld(d_eyef, (128, 128), F32)
            t_ones = ld(d_ones, (1, 128), BF16)
            t_wp = ld(d_wp, (IN, F), BF16)
            t_wo = ld(d_wo, (F, F), BF16)
            t_bp = ld(d_bp, (F, 1), F32)
            t_bo = ld(d_bo, (F, 1), F32)
            t_m1t = [ld(d_m1t[l], (F, H), BF16) for l in range(L)]
            t_m1b = [ld(d_m1b[l], (F, H), BF16) for l in range(L)]
            t_a1t = [ld(d_a1t[l], (F, H), BF16) for l in range(L)]
            t_a1b = [ld(d_a1b[l], (F, H), BF16) for l in range(L)]
            t_m2 = [ld(d_m2[l], (H, H), BF16) for l in range(L)]
            t_u1t = [ld(d_u1t[l], (F, H), BF16) for l in range(L)]
            t_u1b = [ld(d_u1b[l], (H, H), BF16) for l in range(L)]
            t_u2 = [ld(d_u2[l], (H, F), BF16) for l in range(L)]
            t_wa2r = [ld(d_wa2r[l], (128, H), BF16) for l in range(L)]
            t_bm1 = [ld(d_bm1[l], (H, 1), F32) for l in range(L)]
            t_ba1 = [ld(d_ba1[l], (H, 1), F32) for l in range(L)]
            t_bu1 = [ld(d_bu1[l], (H, 1), F32) for l in range(L)]
            t_bu2 = [ld(d_bu2[l], (F, 1), F32) for l in range(L)]

            def bcast4(t):
                # (128,128) tile -> rhs AP (128, [j:0 x4],[h:1 x128])
                a = t[:]
                return AP(t.tensor, a.offset, [list(a.ap[0]), [0, 4], [1, H]])

            def bcast_jh(t):
                # (128,128) tile -> AP (128, [j:0 x128],[h:1 x128])
                a = t[:]
                return AP(t.tensor, a.offset, [list(a.ap[0]), [0, N], [1, H]])

            def as3d(t, nj, nh):
                a = t[:]
                return AP(t.tensor, a.offset, [list(a.ap[0]), [nh, nj], [1, nh]])

            # transpose (128,128) bf16 sbuf -> new bf16 sbuf tile
            def transpose_bf(src_sb, tag):
                p = ps.tile([128, 128], BF16, tag="mm")
                nc.tensor.transpose(p[:], src_sb[:], t_eye[:])
                dst = sm.tile([128, 128], BF16, tag=tag)
                nc.vector.tensor_copy(dst[:], p[:])
                return dst

            for w in range(T):
                # ---- projection: h0^T = Wp^T @ x_w^T + bp
                t_xT = sm.tile([IN, N], BF16, tag="xT")
                nc.sync.dma_start(t_xT[:], d_xT[w])
                p_h = ps.tile([F, N], F32, tag="mm")
                nc.tensor.matmul(p_h[:], t_wp[:], t_xT[:], start=True, stop=True)
                hT_f = sm.tile([F, N], F32, tag="hTf")
                nc.vector.tensor_scalar(hT_f[:], p_h[:], t_bp[:], None, ADD)
                hT_b = sm.tile([F, N], BF16, tag="hTb")
                nc.vector.tensor_copy(hT_b[:], hT_f[:])

                for l in range(L):
                    # ---- small matmuls: A^T, Bm^T, Aa^T, Ba^T (h-part, n-free)
                    p_A = ps.tile([H, N], F32, tag="mm")
                    nc.tensor.matmul(p_A[:], t_m1t[l][:], hT_b[:], start=True, stop=True)
                    ApT = sm.tile([H, N], BF16, tag="ApT")   # A'(=A+bm1)^T (h,i)
                    nc.vector.tensor_scalar(ApT[:], p_A[:], t_bm1[l][:], None, ADD)

                    p_B = ps.tile([H, N], F32, tag="mm")
                    nc.tensor.matmul(p_B[:], t_m1b[l][:], hT_b[:], start=True, stop=True)
                    BmT = sm.tile([H, N], BF16, tag="BmT")   # Bm^T (h,j)
                    nc.vector.tensor_copy(BmT[:], p_B[:])

                    p_Aa = ps.tile([H, N], F32, tag="mm")
                    nc.tensor.matmul(p_Aa[:], t_a1t[l][:], hT_b[:], start=True, stop=True)
                    AaPT = sm.tile([H, N], BF16, tag="AaPT")  # Aa'(=Aa+ba1)^T (h,i)
                    nc.vector.tensor_scalar(AaPT[:], p_Aa[:], t_ba1[l][:], None, ADD)

                    p_Ba = ps.tile([H, N], F32, tag="mm")
                    nc.tensor.matmul(p_Ba[:], t_a1b[l][:], hT_b[:], start=True, stop=True)
                    BaT = sm.tile([H, N], BF16, tag="BaT")   # Ba^T (h,j)
                    nc.vector.tensor_copy(BaT[:], p_Ba[:])

                    # layout prep
                    Aa_i = transpose_bf(AaPT, "Aa_i")        # Aa' (i,h)
                    A_i = transpose_bf(ApT, "A_i")           # A'  (i,h)
                    Ba_j = transpose_bf(BaT, "Ba_j")         # Ba  (j,h)
                    Bm_j = transpose_bf(BmT, "Bm_j")         # Bm  (j,h)
                    Aflat = fl.tile([1, N * H], BF16, tag="Aflat")
                    nc.sync.dma_start(Aflat[:], A_i[:])
                    Baflat = fl.tile([1, N * H], BF16, tag="Baflat")
                    nc.sync.dma_start(Baflat[:], Ba_j[:])

                    # ---- att path: Ra[i,(j,h)] = relu(Aa'[i,h] + Ba[j,h])
                    Ra = big.tile([N, N * H], BF16, tag="Ra")
                    for c in range(NCHUNK):
                        pch = psp.tile([128, CHUNK], F32, tag="prod")
                        nc.tensor.matmul(pch[:], t_eye[:], bcast4(Aa_i),
                                         start=True, stop=False)
                        nc.tensor.matmul(pch[:], t_ones[:],
                                         Baflat[:, c * CHUNK:(c + 1) * CHUNK],
                                         start=False, stop=True)
                        nc.scalar.activation(Ra[:, c * CHUNK:(c + 1) * CHUNK],
                                             pch[:], RELU)
                    # logits = sum_h Ra * wa2
                    tmul = big.tile([N, N * H], BF16, tag="tmul")
                    nc.vector.tensor_tensor(as3d(tmul, N, H), as3d(Ra, N, H),
                                            bcast_jh(t_wa2r[l]), MULT)
                    logits = sm.tile([N, N], F32, tag="logits")
                    nc.vector.tensor_reduce(logits[:], as3d(tmul, N, H), AX, ADD)
                    # masked softmax
                    lm = sm.tile([N, N], F32, tag="lm")
                    nc.vector.tensor_tensor(lm[:], logits[:], t_mask[:], ADD)
                    nmax = sm.tile([N, 1], F32, tag="nmax")
                    nc.vector.tensor_reduce(nmax[:], lm[:], AX, MAX, negate=True)
                    esb = sm.tile([N, N], F32, tag="esb")
                    nc.scalar.activation(esb[:], lm[:], EXP, bias=nmax[:], scale=1.0)
                    ssum = sm.tile([N, 1], F32, tag="ssum")
                    nc.vector.tensor_reduce(ssum[:], esb[:], AX, ADD)
                    rec = sm.tile([N, 1], F32, tag="rec")
                    nc.vector.reciprocal(rec[:], ssum[:])
                    att = sm.tile([N, N], BF16, tag="att")
                    nc.vector.tensor_scalar(att[:], esb[:], rec[:], None, MULT)
                    attT = transpose_bf(att, "attT")         # (j,i)

                    # ---- msg path: Rm[j,(i,h)] = relu(Bm[j,h] + A'[i,h])
                    Rm = big.tile([N, N * H], BF16, tag="Rm")
                    for c in range(NCHUNK):
                        pch = psp.tile([128, CHUNK], F32, tag="prod")
                        nc.tensor.matmul(pch[:], t_eye[:], bcast4(Bm_j),
                                         start=True, stop=False)
                        nc.tensor.matmul(pch[:], t_ones[:],
                                         Aflat[:, c * CHUNK:(c + 1) * CHUNK],
                                         start=False, stop=True)
                        nc.scalar.activation(Rm[:, c * CHUNK:(c + 1) * CHUNK],
                                             pch[:], RELU)
                    # S^T[:, i] = Rm[:, i-block].T @ attT[:, i]
                    pS = ps2.tile([H, N], F32, tag="pS")
                    for i in range(N):
                        nc.tensor.matmul(pS[:, i:i + 1],
                                         Rm[:, i * H:(i + 1) * H],
                                         attT[:, i:i + 1], start=True, stop=True)
                    ST = sm.tile([H, N], BF16, tag="ST")
                    nc.vector.tensor_copy(ST[:], pS[:])

                    # agg^T = Wm2^T @ S^T  (bm2 folded into b_u1 host-side)
                    p_ag = ps.tile([H, N], F32, tag="mm")
                    nc.tensor.matmul(p_ag[:], t_m2[l][:], ST[:], start=True, stop=True)
                    agg = sm.tile([H, N], BF16, tag="agg")
                    nc.vector.tensor_copy(agg[:], p_ag[:])

                    # upd MLP
                    p_u1 = ps.tile([H, N], F32, tag="mm")
                    nc.tensor.matmul(p_u1[:], t_u1t[l][:], hT_b[:], start=True, stop=False)
                    nc.tensor.matmul(p_u1[:], t_u1b[l][:], agg[:], start=False, stop=True)
                    u1 = sm.tile([H, N], BF16, tag="u1")
                    nc.scalar.activation(u1[:], p_u1[:], RELU, bias=t_bu1[l][:])
                    p_up = ps.tile([F, N], F32, tag="mm")
                    nc.tensor.matmul(p_up[:], t_u2[l][:], u1[:], start=True, stop=True)
                    # h += upd + bu2
                    hT_f_new = sm.tile([F, N], F32, tag="hTf")
                    nc.vector.scalar_tensor_tensor(hT_f_new[:], p_up[:], t_bu2[l][:],
                                                   hT_f[:], ADD, ADD)
                    hT_f = hT_f_new
                    hT_b = sm.tile([F, N], BF16, tag="hTb")
                    nc.vector.tensor_copy(hT_b[:], hT_f[:])

                # ---- output proj: out^T = Wo^T @ h^T + bo, then transpose
                p_o = ps.tile([F, N], F32, tag="mm")
                nc.tensor.matmul(p_o[:], t_wo[:], hT_b[:], start=True, stop=True)
                oT = sm.tile([F, N], F32, tag="oT")
                nc.vector.tensor_scalar(oT[:], p_o[:], t_bo[:], None, ADD)
                p_on = ps.tile([N, F], F32, tag="mm")
                nc.tensor.transpose(p_on[:], oT[:], t_eyef[:F, :F])
                o_sb = sm.tile([N, F], F32, tag="o_sb")
                nc.vector.tensor_copy(o_sb[:], p_on[:])
                nc.sync.dma_start(d_out[w], o_sb[:])

    nc.compile()
    return nc


def _prep_in_maps(x, masks, Wp, bp, Wm1, bm1, Wm2, bm2, Wa1, ba1, Wa2, ba2,
                  Wu1, bu1, Wu2, bu2, Wo, bo):
    in_maps = []
    eye = np.eye(128, dtype=np.float32)
    ones1 = np.ones((1, 128), np.float32)
    for c in range(NCORES):
        m = {}
        # x[c]: (N, T, IN) -> (T, IN, N)
        m["xT"] = _bf(np.transpose(x[c], (1, 2, 0)))
        m["maskadd"] = _f32(np.broadcast_to((masks[c] - 1.0) * 3.0e38, (N, N)))
        m["eye"] = _bf(eye)
        m["eyef"] = _f32(eye)
        m["ones1"] = _bf(ones1)
        m["wp"] = _bf(Wp)
        m["wo"] = _bf(Wo)
        m["b_p"] = _f32(bp.reshape(F, 1))
        m["b_o"] = _f32(bo.reshape(F, 1))
        for l in range(L):
            m[f"w_m1t{l}"] = _bf(Wm1[l][:F])
            m[f"w_m1b{l}"] = _bf(Wm1[l][F:])
            m[f"w_a1t{l}"] = _bf(Wa1[l][:F])
            m[f"w_a1b{l}"] = _bf(Wa1[l][F:])
            m[f"w_m2_{l}"] = _bf(Wm2[l])
            m[f"w_u1t{l}"] = _bf(Wu1[l][:F])
            m[f"w_u1b{l}"] = _bf(Wu1[l][F:])
            m[f"w_u2_{l}"] = _bf(Wu2[l])
            m[f"wa2r{l}"] = _bf(np.broadcast_to(Wa2[l][:, 0], (128, H)))
            m[f"b_m1_{l}"] = _f32(bm1[l].reshape(H, 1))
            m[f"b_a1_{l}"] = _f32(ba1[l].reshape(H, 1))
            # fold bm2 @ Wu1_bot into bu1 (softmax rows sum to 1)
            m[f"b_u1_{l}"] = _f32((bu1[l] + bm2[l] @ Wu1[l][F:]).reshape(H, 1))
            m[f"b_u2_{l}"] = _f32(bu2[l].reshape(F, 1))
        in_maps.append(m)
    return in_maps


def kernel(**inputs) -> np.ndarray:
    from concourse.bass_utils import run_bass_kernel_spmd
    global _CACHED_NC
    if _CACHED_NC is None:
        _CACHED_NC = _build_nc()
    nc = _CACHED_NC
    args = {k: np.asarray(v) for k, v in inputs.items()}
    in_maps = _prep_in_maps(**args)
    res = run_bass_kernel_spmd(nc, in_maps, list(range(NCORES)))
    # out per core: (T, N, F) -> full (B, N, T, F)
    out = np.stack([np.transpose(np.asarray(res.results[c]["out"], np.float32),
                                 (1, 0, 2)) for c in range(NCORES)])
    return out


# revision 11
# speedup vs baseline: 8.0595x; 8.0595x over previous
"""Trainium2 Bass kernel for EnhancedTrajectoryPredictor GNN message passing.

Data-parallel over batch: core c handles batch element c (T=4 windows each).
Per window-layer, the two O(N^2*H) pairwise relu tensors are produced on the
PE via identity/ones broadcast matmuls in a shared (j-part,(i,h)) layout and
relu-drained on ACT. Attention logits are computed transposed — |wa2| is
folded into Wa1 host-side with the h-dim permuted by sign(wa2), so the
contraction is two strided range-reductions and a subtract; the mask folds
into the exp bias; normalization folds into the S drain (unnormalized
attention feeds per-i PE matmuls). The softmax row-sum==1 identity
eliminates the O(N^2*H*H) message matmul. All weights ship in two packed
blob parameters to minimize I/O buffer count.
"""
import numpy as np
import ml_dtypes

import concourse.bass as bass
import concourse.mybir as mybir
import concourse.tile as tile
from concourse import bacc
from concourse.bass_types import AP

F32 = mybir.dt.float32
BF16 = mybir.dt.bfloat16

B, N, T, IN = 8, 128, 4, 45
F, H, L = 64, 128, 3
NCORES = 8
CHUNK = 1024               # psum chunk free size (i-block of 8)
NCHUNK = (N * H) // CHUNK  # 16
JB = CHUNK // H            # 8

_bf = lambda x: np.ascontiguousarray(x).astype(ml_dtypes.bfloat16)
_f32 = lambda x: np.ascontiguousarray(x).astype(np.float32)

_CACHE = {}


class _Packer:
    def __init__(self):
        self.off = 0
        self.items = {}

    def add(self, name, rows, cols):
        self.items[name] = (0, rows, self.off, cols)
        self.off += cols
        return self.items[name]


def _layout():
    """bf16 blob layout: (128, Wb) ; f32 blob layout: (128, Wf)."""
    pb = _Packer()
    pb.add("eye", 128, 128)
    pb.add("ones1", 1, 128)
    pb.add("onesK", 128, 1)
    pb.add("wp", IN, F)
    pb.add("wo", F, F)
    for l in range(L):
        pb.add(f"m1t{l}", F, H)
        pb.add(f"m1b{l}", F, H)
        pb.add(f"a1t{l}", F, H)   # |wa2|-scaled, sign-permuted
        pb.add(f"a1b{l}", F, H)   # |wa2|-scaled, sign-permuted
        pb.add(f"m2{l}", H, H)
        pb.add(f"u1t{l}", F, H)
        pb.add(f"u1b{l}", H, H)
        pb.add(f"u2{l}", H, F)
    pf = _Packer()
    pf.add("maskcol", N, 1)
    pf.add("eyef", F, F)
    pf.add("bp", F, 1)
    pf.add("bo", F, 1)
    for l in range(L):
        pf.add(f"bm1r{l}", 128, H)   # bm1 row replicated
        pf.add(f"ba1r{l}", 128, H)   # |wa2|-scaled ba1, sign-permuted, replicated
        pf.add(f"bu1{l}", H, 1)      # bu1 + bm2 @ Wu1_bot
        pf.add(f"bu2{l}", F, 1)
    return pb, pf


_PB, _PF = _layout()


def _build_nc(p_split):
    """p_split[l] = number of wa2>=0 columns (h-permutation puts them first)."""
    nc = bacc.Bacc(None, target_bir_lowering=False, debug=False)

    d_xT = nc.declare_dram_parameter("xT", [T, IN, N], BF16, isOutput=False)
    d_wb = nc.declare_dram_parameter("wb", [128, _PB.off], BF16, isOutput=False)
    d_wf = nc.declare_dram_parameter("wf", [128, _PF.off], F32, isOutput=False)
    d_out = nc.declare_dram_parameter("out", [T, N, F], F32, isOutput=True)

    RELU = mybir.ActivationFunctionType.Relu
    EXP = mybir.ActivationFunctionType.Exp
    ADD = mybir.AluOpType.add
    SUB = mybir.AluOpType.subtract
    MULT = mybir.AluOpType.mult
    AX = mybir.AxisListType.X

    with tile.TileContext(nc) as tc:
        with (
            tc.tile_pool(name="wts", bufs=1) as wts,
            tc.tile_pool(name="sm", bufs=3) as sm,
            tc.tile_pool(name="big", bufs=2) as big,
            tc.tile_pool(name="fl", bufs=1) as fl,
            tc.tile_pool(name="ps", bufs=2, space="PSUM") as ps,
            tc.tile_pool(name="ps2", bufs=1, space="PSUM") as ps2,
            tc.tile_pool(name="psp", bufs=2, space="PSUM") as psp,
        ):
            t_wb = wts.tile([128, _PB.off], BF16, tag="wb")
            nc.sync.dma_start(t_wb[:], d_wb[:])
            t_wf = wts.tile([128, _PF.off], F32, tag="wf")
            nc.sync.dma_start(t_wf[:], d_wf[:])

            def wb(name):
                r0, nr, c0, ncol = _PB.items[name]
                return t_wb[r0:r0 + nr, c0:c0 + ncol]

            def wf(name):
                r0, nr, c0, ncol = _PF.items[name]
                return t_wf[r0:r0 + nr, c0:c0 + ncol]

            def bcastH(ap_):
                # (128,H) AP -> (128, [i:0 x 4],[h:1 x H])  (512-wide half-chunk)
                return AP(ap_.tensor, ap_.offset,
                          [list(ap_.ap[0]), [0, 4], [1, H]])

            a_eye = wb("eye")
            a_ones = wb("ones1")
            a_onesK = wb("onesK")

            for w in range(T):
                # ---- projection: h0^T = Wp^T @ x_w^T + bp
                t_xT = sm.tile([IN, N], BF16, tag="xT")
                nc.sync.dma_start(t_xT[:], d_xT[w])
                p_h = ps.tile([H, N], F32, tag="mm")
                nc.tensor.matmul(p_h[:F, :], wb("wp"), t_xT[:], start=True, stop=True)
                hT_f = sm.tile([F, N], F32, tag="hTf")
                nc.vector.tensor_scalar(hT_f[:], p_h[:F, :], wf("bp"), None, ADD)
                hT_b = sm.tile([F, N], BF16, tag="hTb")
                nc.vector.tensor_copy(hT_b[:], hT_f[:])

                for l in range(L):
                    p = p_split[l]
                    # ---- prep matmuls: lhsT = h^T, rhs = W halves
                    p_A = ps.tile([H, N], F32, tag="mm")
                    nc.tensor.matmul(p_A[:], hT_b[:], wb(f"m1t{l}"), start=True, stop=True)
                    p_Aa = ps.tile([H, N], F32, tag="mm")
                    nc.tensor.matmul(p_Aa[:], hT_b[:], wb(f"a1t{l}"), start=True, stop=True)
                    # combined (i,h) tile for single flat DMA: [A | Aa]
                    AB_sb = sm.tile([N, 2 * H], BF16, tag="AB_sb")
                    nc.vector.tensor_copy(AB_sb[:, 0:H], p_A[:])
                    nc.vector.tensor_copy(AB_sb[:, H:2 * H], p_Aa[:])

                    p_Bm = ps.tile([H, N], F32, tag="mm")
                    nc.tensor.matmul(p_Bm[:], hT_b[:], wb(f"m1b{l}"), start=True, stop=True)
                    Bm_sb = sm.tile([N, H], BF16, tag="Bm_sb")   # Bm + bm1
                    nc.vector.tensor_tensor(Bm_sb[:], p_Bm[:], wf(f"bm1r{l}"), ADD)

                    p_Ba = ps.tile([H, N], F32, tag="mm")
                    nc.tensor.matmul(p_Ba[:], hT_b[:], wb(f"a1b{l}"), start=True, stop=True)
                    Ba_sb = sm.tile([N, H], BF16, tag="Ba_sb")   # Ba + ba1 (scaled)
                    nc.vector.tensor_tensor(Ba_sb[:], p_Ba[:], wf(f"ba1r{l}"), ADD)

                    # flat of [A | Aa]: flatAB[i*256 + s*128 + h]; 4-way split DMA
                    flatAB = fl.tile([1, N * 2 * H], BF16, tag="flatAB")
                    for k in range(4):
                        eng = nc.sync if k % 2 == 0 else nc.gpsimd
                        eng.dma_start(flatAB[:, k * 32 * 256:(k + 1) * 32 * 256],
                                      AB_sb[k * 32:(k + 1) * 32, :])

                    def flat_rhs2(sel, c, hh):
                        # rhs (1, [i: 4 x 256],[h: 128]) for half-chunk (c, hh)
                        a = flatAB[:]
                        return AP(flatAB.tensor,
                                  a.offset + (c * JB + hh * 4) * 256 + sel * 128,
                                  [list(a.ap[0]), [256, 4], [1, H]])

                    # ---- att: Ra[j,(i,h)] = relu(Ba'[j,h] + Aa[i,h]); consume chunked
                    logitsT = sm.tile([N, N], F32, tag="logitsT")  # (j,i)
                    for c in range(NCHUNK):
                        pch = psp.tile([128, CHUNK], F32, tag="prod")
                        for hh in range(2):
                            sl = slice(hh * 512, (hh + 1) * 512)
                            nc.tensor.matmul(pch[:, sl], a_eye, bcastH(Ba_sb[:]),
                                             start=True, stop=False)
                            nc.tensor.matmul(pch[:, sl], a_ones, flat_rhs2(1, c, hh),
                                             start=False, stop=True)
                        rach = sm.tile([128, CHUNK], BF16, tag="rach")
                        nc.scalar.activation(rach[:], pch[:], RELU)
                        ra = rach[:]
                        if 0 < p < H:
                            tpos = sm.tile([128, JB], F32, tag="tpos")
                            nc.vector.tensor_reduce(
                                tpos[:], AP(rach.tensor, ra.offset,
                                            [list(ra.ap[0]), [H, JB], [1, p]]),
                                AX, ADD)
                            tneg = sm.tile([128, JB], F32, tag="tneg")
                            nc.vector.tensor_reduce(
                                tneg[:], AP(rach.tensor, ra.offset + p,
                                            [list(ra.ap[0]), [H, JB], [1, H - p]]),
                                AX, ADD)
                            nc.vector.tensor_tensor(
                                logitsT[:, c * JB:(c + 1) * JB], tpos[:], tneg[:], SUB)
                        else:
                            sgn = 1.0 if p == H else -1.0
                            tpos = sm.tile([128, JB], F32, tag="tpos")
                            nc.vector.tensor_reduce(
                                tpos[:], AP(rach.tensor, ra.offset,
                                            [list(ra.ap[0]), [H, JB], [1, H]]),
                                AX, ADD)
                            nc.vector.tensor_scalar(
                                logitsT[:, c * JB:(c + 1) * JB], tpos[:], sgn, None, MULT)
                    # expT = exp(logitsT + mask[j]) -- unnormalized att^T (j,i)
                    expT = sm.tile([N, N], BF16, tag="expT")
                    nc.scalar.activation(expT[:], logitsT[:], EXP,
                                         bias=wf("maskcol"), scale=1.0)
                    # denominators: colsum over j via ones matvec
                    p_den = ps2.tile([1, N], F32, tag="pden")
                    nc.tensor.matmul(p_den[:], a_onesK, expT[:], start=True, stop=True)
                    rec = sm.tile([1, N], F32, tag="rec")
                    nc.vector.reciprocal(rec[:], p_den[:])
                    rec_rep = sm.tile([128, N], F32, tag="rec_rep")
                    nc.gpsimd.partition_broadcast(rec_rep[:], rec[:])

                    # ---- msg: Rm[j,(i,h)] = relu(Bm'[j,h] + A[i,h])
                    Rm = big.tile([N, N * H], BF16, tag="Rm")
                    for c in range(NCHUNK):
                        pch = psp.tile([128, CHUNK], F32, tag="prod")
                        for hh in range(2):
                            sl = slice(hh * 512, (hh + 1) * 512)
                            nc.tensor.matmul(pch[:, sl], a_eye, bcastH(Bm_sb[:]),
                                             start=True, stop=False)
                            nc.tensor.matmul(pch[:, sl], a_ones, flat_rhs2(0, c, hh),
                                             start=False, stop=True)
                        nc.scalar.activation(Rm[:, c * CHUNK:(c + 1) * CHUNK],
                                             pch[:], RELU)
                    # S^T[:, i] = Rm[:, i-block].T @ expT[:, i]  (unnormalized)
                    pS = ps2.tile([H, N], F32, tag="pS")
                    for i in range(N):
                        nc.tensor.matmul(pS[:, i:i + 1],
                                         Rm[:, i * H:(i + 1) * H],
                                         expT[:, i:i + 1], start=True, stop=True)
                    # drain + normalize by 1/den[i]
                    ST = sm.tile([H, N], BF16, tag="ST")
                    nc.vector.tensor_tensor(ST[:], pS[:], rec_rep[:], MULT)

                    # agg^T = Wm2^T @ S^T  (bm2 folded into bu1 host-side)
                    p_ag = ps.tile([H, N], F32, tag="mm")
                    nc.tensor.matmul(p_ag[:], wb(f"m2{l}"), ST[:], start=True, stop=True)
                    agg = sm.tile([H, N], BF16, tag="agg")
                    nc.vector.tensor_copy(agg[:], p_ag[:])

                    # upd MLP
                    p_u1 = ps.tile([H, N], F32, tag="mm")
                    nc.tensor.matmul(p_u1[:], wb(f"u1t{l}"), hT_b[:], start=True, stop=False)
                    nc.tensor.matmul(p_u1[:], wb(f"u1b{l}"), agg[:], start=False, stop=True)
                    u1 = sm.tile([H, N], BF16, tag="u1")
                    nc.scalar.activation(u1[:], p_u1[:], RELU, bias=wf(f"bu1{l}"))
                    p_up = ps.tile([H, N], F32, tag="mm")
                    nc.tensor.matmul(p_up[:F, :], wb(f"u2{l}"), u1[:], start=True, stop=True)
                    # h += upd + bu2
                    hT_f_new = sm.tile([F, N], F32, tag="hTf")
                    nc.vector.scalar_tensor_tensor(hT_f_new[:], p_up[:F, :],
                                                   wf(f"bu2{l}"), hT_f[:], ADD, ADD)
                    hT_f = hT_f_new
                    hT_b = sm.tile([F, N], BF16, tag="hTb")
                    nc.vector.tensor_copy(hT_b[:], hT_f[:])

                # ---- output proj: out^T = Wo^T @ h^T + bo, then transpose
                p_o = ps.tile([H, N], F32, tag="mm")
                nc.tensor.matmul(p_o[:F, :], wb("wo"), hT_b[:], start=True, stop=True)
                oT = sm.tile([F, N], F32, tag="oT")
                nc.vector.tensor_scalar(oT[:], p_o[:F, :], wf("bo"), None, ADD)
                p_on = ps.tile([H, N], F32, tag="mm")
                nc.tensor.transpose(p_on[:N, :F], oT[:], wf("eyef"))
                o_sb = sm.tile([N, F], F32, tag="o_sb")
                nc.vector.tensor_copy(o_sb[:], p_on[:N, :F])
                nc.sync.dma_start(d_out[w], o_sb[:])

    nc.compile()
    return nc


def _pack_blobs(core_mask, Wp, bp, Wm1, bm1, Wm2, bm2, Wa1, ba1, Wa2, ba2,
                Wu1, bu1, Wu2, bu2, Wo, bo, perms):
    wb_blob = np.zeros((128, _PB.off), np.float32)
    wf_blob = np.zeros((128, _PF.off), np.float32)

    def putb(name, v):
        r0, nr, c0, ncol = _PB.items[name]
        wb_blob[r0:r0 + nr, c0:c0 + ncol] = v

    def putf(name, v):
        r0, nr, c0, ncol = _PF.items[name]
        wf_blob[r0:r0 + nr, c0:c0 + ncol] = v

    putb("eye", np.eye(128, dtype=np.float32))
    putb("ones1", np.ones((1, 128), np.float32))
    putb("onesK", np.ones((128, 1), np.float32))
    putb("wp", Wp)
    putb("wo", Wo)
    putf("maskcol", ((core_mask - 1.0) * 3.0e38).reshape(N, 1))
    putf("eyef", np.eye(F, dtype=np.float32))
    putf("bp", bp.reshape(F, 1))
    putf("bo", bo.reshape(F, 1))
    for l in range(L):
        perm, scale = perms[l]
        putb(f"m1t{l}", Wm1[l][:F])
        putb(f"m1b{l}", Wm1[l][F:])
        putb(f"a1t{l}", Wa1[l][:F][:, perm] * scale[None, :])
        putb(f"a1b{l}", Wa1[l][F:][:, perm] * scale[None, :])
        putb(f"m2{l}", Wm2[l])
        putb(f"u1t{l}", Wu1[l][:F])
        putb(f"u1b{l}", Wu1[l][F:])
        putb(f"u2{l}", Wu2[l])
        putf(f"bm1r{l}", np.broadcast_to(bm1[l], (128, H)))
        putf(f"ba1r{l}", np.broadcast_to(ba1[l][perm] * scale, (128, H)))
        putf(f"bu1{l}", (bu1[l] + bm2[l] @ Wu1[l][F:]).reshape(H, 1))
        putf(f"bu2{l}", bu2[l].reshape(F, 1))
    return wb_blob, wf_blob


def prepare(**inputs):
    args = {k: np.asarray(v) for k, v in inputs.items()}
    x, masks = _f32(args["x"]), _f32(args["masks"])
    Wa2 = _f32(args["Wa2"])

    # sign-split permutation per layer: wa2>=0 columns first, |wa2| folded in
    perms, p_split = [], []
    for l in range(L):
        wa2 = Wa2[l][:, 0]
        order = np.argsort(~(wa2 >= 0), kind="stable")  # positives first
        perms.append((order, np.abs(wa2)[order]))
        p_split.append(int((wa2 >= 0).sum()))

    key = tuple(p_split)
    if key not in _CACHE:
        _CACHE[key] = _build_nc(p_split)
    nc = _CACHE[key]

    wkeys = dict(Wp=args["Wp"], bp=args["bp"], Wm1=args["Wm1"], bm1=args["bm1"],
                 Wm2=args["Wm2"], bm2=args["bm2"], Wa1=args["Wa1"], ba1=args["ba1"],
                 Wa2=Wa2, ba2=args["ba2"], Wu1=args["Wu1"], bu1=args["bu1"],
                 Wu2=args["Wu2"], bu2=args["bu2"], Wo=args["Wo"], bo=args["bo"])
    in_maps = []
    for c in range(NCORES):
        wb_blob, wf_blob = _pack_blobs(masks[c], perms=perms, **wkeys)
        in_maps.append({
            "xT": _bf(np.transpose(x[c], (1, 2, 0))),
            "wb": _bf(wb_blob),
            "wf": _f32(wf_blob),
        })
    return nc, in_maps


def kernel(**inputs) -> np.ndarray:
    from concourse.bass_utils import run_bass_kernel_spmd
    nc, in_maps = prepare(**inputs)
    res = run_bass_kernel_spmd(nc, in_maps, list(range(NCORES)))
    out = np.stack([np.transpose(np.asarray(res.results[c]["out"], np.float32),
                                 (1, 0, 2)) for c in range(NCORES)])
    return out
